# revision 7
# baseline (speedup 1.0000x reference)
"""GAT node encoder (3 GATConv+BN layers) on 8 trn2 NeuronCores — v2.

Sharding: nodes partitioned across cores (dst-sharded message passing).
Per layer, per core:
  1. AllGather of the (bf16) layer input y^T across cores  (layer 0: full x
     is staged to every core at setup, so no collective at all)
  2. full-table matmul (redundant on every core): every core computes the
     whole table h_tab[n] = [y[n]@W | s[n] (fp32 bits)] (bf16 rows, local
     DRAM) and a block-replicated d table dtab[loc, blk] = y[n]@W@a_dst.
  3. edge phase: edges grouped into 128-edge tiles per 128-dst-node tile,
     columns split into lo/hi regions by source row (dma_gather int16 range).
     Per 16-column batch: dma_gather of h|s rows, dma_gather of d rows,
     p = exp(leakyrelu(s+d)) (no max needed: |e| stays small), in-place
     alpha-scale of h, one-hot dst masks on DVE. Segment-sum runs on the
     tensor engine: psA[d,:] += sum_s mask[s,d]*(h[s,:]*p[s,h]),
     psD[d,h] += sum_s mask[s,d]*p[s,h]; out = psA/psD.
  4. BatchNorm: feature-major stats + tiny AllReduce, fused scale/shift.

The per-feature bias b is dropped: BN(o + b) == BN(o) exactly. The 1/H head
mean is dropped: BN is scale-invariant per feature.
"""
import sys

sys.path.insert(0, "/opt/trn_rl_repo")

import numpy as np

import concourse.bass as bass
import concourse.bacc as bacc
import concourse.tile as tile
from concourse import mybir
from concourse import bass_utils
from concourse.masks import make_identity

NCORES = 8
P = 128
NEG_SLOPE = 0.2
EPS_BN = 1e-5
GB = 8         # edge-tile columns per gather batch (dma_gather <=1024 idxs)
EV = 8         # mm tiles per evacuation batch
LOHI = 32768   # int16 index range split
DREP = 64      # dtab row elems (f32): NCORES*H padded to 256B

F32 = mybir.dt.float32
BF16 = mybir.dt.bfloat16
I16 = mybir.dt.int16


def _row_elems(HC, H):
    """bf16 elems per h|s table row, padded to a 256-byte multiple."""
    return (HC + 2 * H + 127) // 128 * 128


def _wrap16(flat):
    """dma_gather index layout: [128, n/16] int16, tile[p, s] = flat[s*16+p%16]."""
    n = len(flat)
    assert n % 16 == 0
    w = np.zeros((P, n // 16), np.int16)
    w[:16] = np.asarray(flat, np.int16).reshape(-1, 16).T
    for g in range(1, 8):
        w[g * 16:(g + 1) * 16] = w[:16]
    return w


# ----------------------------------------------------------------------------
# host-side graph preprocessing
# ----------------------------------------------------------------------------

def _prep(edge_index, N):
    src = np.asarray(edge_index[0], dtype=np.int64)
    dst = np.asarray(edge_index[1], dtype=np.int64)
    loops = np.arange(N, dtype=np.int64)
    src = np.concatenate([src, loops])
    dst = np.concatenate([dst, loops])

    shard = N // NCORES
    ntiles = (shard + P) // P                # >= 1 pad row per shard
    shard_pad = ntiles * P
    nrows = NCORES * shard_pad

    deg = np.bincount(dst, minlength=N)
    node_row = np.empty(N, np.int64)         # orig node -> global table row
    out_nodes = []                           # core -> orig node per local row
    for c in range(NCORES):
        lo = c * shard
        nodes = np.arange(lo, lo + shard)
        order = np.argsort(-deg[lo:lo + shard], kind="stable")
        nodes = nodes[order]
        node_row[nodes] = c * shard_pad + np.arange(shard)
        out_nodes.append(nodes)

    srow = node_row[src]                     # src global row
    drow = node_row[dst]                     # dst global row
    c_of = drow // shard_pad
    r_of = drow % shard_pad
    t_of = r_of // P
    hi_of = (srow >= LOHI).astype(np.int64)  # region of the source row

    # per (core, dst-tile, region) counts -> uniform K_lo/K_hi
    cnt = np.zeros((NCORES, ntiles, 2), np.int64)
    np.add.at(cnt, (c_of, t_of, hi_of), 1)
    cmax = cnt.max(axis=0)                   # [ntiles, 2]
    K_lo = np.maximum(1, (cmax[:, 0] + P - 1) // P)
    K_hi = np.maximum(1, (cmax[:, 1] + P - 1) // P)
    lo_offs = np.zeros(ntiles + 1, np.int64)
    lo_offs[1:] = np.cumsum(K_lo)
    NLO = int(lo_offs[-1])
    hi_offs = np.zeros(ntiles + 1, np.int64)
    hi_offs[1:] = np.cumsum(K_hi)
    NHI = int(hi_offs[-1])
    NTE = NLO + NHI

    # pad rows (nodes with s = -1e30): block0 pad for lo, block7 pad for hi
    pad_lo = 0 * shard_pad + shard
    pad_hi = (NCORES - 1) * shard_pad + shard
    assert pad_lo < LOHI and pad_hi >= LOHI

    hidx = np.full((NCORES, P, NTE), 0, np.int64)
    hidx[:, :, :NLO] = pad_lo
    hidx[:, :, NLO:] = pad_hi
    didx = np.full((NCORES, P, NTE), shard, np.int64)   # local pad row
    dstloc = np.zeros((NCORES, P, NTE), np.float32)
    for c in range(NCORES):
        m = c_of == c
        sg, rr, tt, hh = srow[m], r_of[m], t_of[m], hi_of[m]
        order = np.lexsort((rr, hh, tt))
        sg, rr, tt, hh = sg[order], rr[order], tt[order], hh[order]
        # position within each (tile, region) run
        run_key = tt * 2 + hh
        first = np.r_[True, run_key[1:] != run_key[:-1]]
        starts = np.flatnonzero(first)
        run_id = np.cumsum(first) - 1
        q = np.arange(len(tt)) - starts[run_id]
        col = np.where(hh == 0, lo_offs[tt], NLO + hi_offs[tt]) + q // P
        part = q % P
        hidx[c, part, col] = sg
        didx[c, part, col] = rr
        dstloc[c, part, col] = (rr % P).astype(np.float32)

    # wrapped int16 index arrays (column-major flattening: flat[j*128+p])
    hflat = np.empty((NCORES, NTE * P), np.int64)
    dflat = np.empty((NCORES, NTE * P), np.int64)
    for c in range(NCORES):
        hflat[c] = hidx[c].T.ravel()
        dflat[c] = didx[c].T.ravel()
    hflat[:, NLO * P:] -= LOHI
    h16 = np.stack([_wrap16(hflat[c]) for c in range(NCORES)])
    d16 = np.stack([_wrap16(dflat[c]) for c in range(NCORES)])

    return {
        "shard": shard, "shard_pad": shard_pad, "ntiles": ntiles,
        "nrows": nrows, "NTE": NTE, "NLO": NLO, "NHI": NHI,
        "K_lo": K_lo.astype(int).tolist(), "K_hi": K_hi.astype(int).tolist(),
        "lo_offs": lo_offs.astype(int).tolist(),
        "hi_offs": hi_offs.astype(int).tolist(),
        "node_row": node_row, "out_nodes": out_nodes,
        "h16": h16, "d16": d16, "dstloc": dstloc,
    }


# ----------------------------------------------------------------------------
# device program
# ----------------------------------------------------------------------------

def _build_program(g, layers, in_dim, ablate=()):
    """layers: list of dicts {H, C} per layer."""
    ablate = set(ablate)
    shard, shard_pad, ntiles = g["shard"], g["shard_pad"], g["ntiles"]
    nrows, NTE, NLO = g["nrows"], g["NTE"], g["NLO"]
    K_lo, K_hi = g["K_lo"], g["K_hi"]
    lo_offs, hi_offs = g["lo_offs"], g["hi_offs"]
    n_l = len(layers)
    HMAX = max(L["H"] for L in layers)

    nc = bacc.Bacc("TRN2", target_bir_lowering=False, debug=False, num_devices=NCORES,
                   num_swdge_queues=2)

    xT = nc.dram_tensor("xT", [in_dim, shard_pad], BF16, kind="ExternalInput").ap()
    h16_in = nc.dram_tensor("h16", [16, NTE * 8], I16, kind="ExternalInput").ap()
    d16_in = nc.dram_tensor("d16", [16, NTE * 8], I16, kind="ExternalInput").ap()
    dl_in = nc.dram_tensor("dstloc", [P, NTE], mybir.dt.int8, kind="ExternalInput").ap()
    sel_ins = [nc.dram_tensor(f"sel{l}", [P, DREP], F32, kind="ExternalInput").ap()
               for l in range(n_l)]
    wexts = []
    for l, L in enumerate(layers):
        Kdim = in_dim if l == 0 else layers[l - 1]["C"]
        wexts.append(nc.dram_tensor(
            f"wext{l}", [Kdim, L["H"] * L["C"] + 2 * L["H"]],
            BF16, kind="ExternalInput").ap())
    gb = nc.dram_tensor("gb", [P, 2 * n_l], F32, kind="ExternalInput").ap()
    spad_in = nc.dram_tensor("spad", [P, 1], F32, kind="ExternalInput").ap()
    out_t = nc.dram_tensor("out", [P, shard_pad], BF16, kind="ExternalOutput").ap()

    with tile.TileContext(nc) as tc:
        import contextlib
        with contextlib.ExitStack() as ctx:
            dram = ctx.enter_context(tc.tile_pool(name="dram", bufs=1, space="DRAM"))
            psA = ctx.enter_context(tc.tile_pool(name="psA", bufs=2, space="PSUM"))
            psB = ctx.enter_context(tc.tile_pool(name="psB", bufs=2, space="PSUM"))
            psT = ctx.enter_context(tc.tile_pool(name="psT", bufs=2, space="PSUM"))
            sb = ctx.enter_context(tc.tile_pool(name="sb", bufs=1))
            lhsp = ctx.enter_context(tc.tile_pool(name="lhsp", bufs=3))
            evh = ctx.enter_context(tc.tile_pool(name="evh", bufs=2))
            evs = ctx.enter_context(tc.tile_pool(name="evs", bufs=2))
            hgp = ctx.enter_context(tc.tile_pool(name="hgp", bufs=4))
            dgp = ctx.enter_context(tc.tile_pool(name="dgp", bufs=3))
            mkp = ctx.enter_context(tc.tile_pool(name="mkp", bufs=4))
            etp = ctx.enter_context(tc.tile_pool(name="etp", bufs=3))
            onp = ctx.enter_context(tc.tile_pool(name="onp", bufs=2))
            smp = ctx.enter_context(tc.tile_pool(name="smp", bufs=8))
            wkp = ctx.enter_context(tc.tile_pool(name="wkp", bufs=2))

            ident = sb.tile([P, P], F32, tag="ident")
            make_identity(nc, ident[:])
            iota_i = sb.tile([P, P], mybir.dt.int32, tag="iota_i")
            nc.gpsimd.iota(iota_i[:], pattern=[[1, P]], base=0, channel_multiplier=0)
            iota_f = sb.tile([P, P], BF16, tag="iota_f")
            nc.vector.tensor_copy(iota_f[:], iota_i[:])
            h16_t = sb.tile([P, NTE * 8], I16, tag="h16")
            d16_t = sb.tile([P, NTE * 8], I16, tag="d16")
            for gg in range(8):
                nc.sync.dma_start(h16_t[gg * 16:(gg + 1) * 16, :], h16_in[:])
                nc.sync.dma_start(d16_t[gg * 16:(gg + 1) * 16, :], d16_in[:])
            dl_i = sb.tile([P, NTE], mybir.dt.int8, tag="dli")
            nc.sync.dma_start(dl_i[:], dl_in[:])
            dl_t = sb.tile([P, NTE], BF16, tag="dl")
            nc.vector.tensor_copy(dl_t[:], dl_i[:])
            sel_ts = []
            for l in range(n_l):
                s_t = sb.tile([P, DREP], F32, tag=f"sel{l}")
                nc.sync.dma_start(s_t[:], sel_ins[l][:])
                sel_ts.append(s_t)
            gb_t = sb.tile([P, 2 * n_l], F32, tag="gb")
            nc.sync.dma_start(gb_t[:], gb[:])
            spad_t = sb.tile([P, 1], F32, tag="spad")
            nc.sync.dma_start(spad_t[:], spad_in[:])

            yT = None  # [P(feat), shard_pad] f32 SBUF, input to next layer
            for l, L in enumerate(layers):
                H, C = L["H"], L["C"]
                HC = H * C
                SD = 2 * H
                RB = _row_elems(HC, H)       # bf16 elems per table row
                Kdim = in_dim if l == 0 else layers[l - 1]["C"]
                kchunks = Kdim // P
                ldt = BF16

                # ---- phase 0: AllGather the layer input across cores ----
                with nc.named_scope(f"ag{l}"):
                    if l == 0:
                        ag_in = dram.tile([in_dim, shard_pad], BF16, tag="agin0")
                        nc.sync.dma_start(ag_in[:], xT[:])
                    else:
                        yTb = sb.tile([P, shard_pad], BF16, tag="yTb")
                        nc.vector.tensor_copy(yTb[:], yT[:])
                        ag_in = dram.tile([P, shard_pad], BF16, tag=f"agin{l}")
                        nc.sync.dma_start(ag_in[:], yTb[:])
                    y_ag = dram.tile([NCORES * (in_dim if l == 0 else P), shard_pad],
                                     BF16, tag=f"yag{l}", addr_space="Shared")
                    if "coll" not in ablate:
                        nc.gpsimd.collective_compute(
                            "AllGather", mybir.AluOpType.bypass,
                            replica_groups=[list(range(NCORES))],
                            ins=[ag_in.opt()], outs=[y_ag.opt()],
                        )

                # ---- phase 1: full-table matmul -> h_tab (h|s bf16), dtab (d f32) ----
                ctx_mm = nc.named_scope(f"mm{l}"); ctx_mm.__enter__()
                h_tab = dram.tile([nrows, RB], BF16, tag=f"htab{l % 2}")
                dtab = dram.tile([shard_pad, DREP], F32, tag=f"dtab{l % 2}")

                wk = []
                for k in range(kchunks):
                    w = wkp.tile([P, HC + SD], ldt, tag="wext")
                    nc.sync.dma_start(w[:], wexts[l][k * P:(k + 1) * P, :])
                    wk.append(w)

                for blk in range(NCORES):
                    for t0 in range(0, ntiles, EV):
                        ts = min(EV, ntiles - t0)
                        lds = []
                        for k in range(kchunks):
                            ld = lhsp.tile([P, EV * P], ldt, tag="ld")
                            rb_ = blk * Kdim + k * P
                            src_slice = y_ag[rb_:rb_ + P, t0 * P:t0 * P + ts * P]
                            nc.sync.dma_start(ld[:, :ts * P], src_slice)
                            lds.append(ld)
                        hstg = evh.tile([P, EV * RB], BF16, tag="hstg")
                        dstg = evs.tile([P, EV * HMAX], F32, tag="dstg")
                        psd = psB.tile([P, EV * 2 * HMAX], F32, tag="sd", space="PSUM")
                        for ti in range(ts):
                            ph = psA.tile([P, 512], F32, tag="acc", space="PSUM")
                            for k in range(kchunks if "mm" not in ablate else 0):
                                lhs_ap = lds[k][:, ti * P:(ti + 1) * P]
                                nc.tensor.matmul(ph[:, :HC], lhsT=lhs_ap,
                                                 rhs=wk[k][:, :HC],
                                                 start=(k == 0), stop=(k == kchunks - 1))
                                nc.tensor.matmul(psd[:, ti * SD:(ti + 1) * SD], lhsT=lhs_ap,
                                                 rhs=wk[k][:, HC:HC + SD],
                                                 start=(k == 0), stop=(k == kchunks - 1))
                            nc.scalar.copy(hstg[:, ti * RB:ti * RB + HC], ph[:, :HC])
                        # s (fp32 bits inside the bf16 row) and d, for the whole group
                        hsv = hstg[:].rearrange("p (j q) -> p j q", j=EV)[
                            :, :ts, HC:HC + SD].bitcast(F32)      # [P, ts, H]
                        pdv = psd[:, :ts * SD].rearrange("p (j q) -> p j q", q=SD)
                        nc.vector.tensor_copy(hsv[:, :, :], pdv[:, :, :H])
                        if t0 + ts == ntiles:
                            # last group of this block: pad-node s += -1e30
                            nc.vector.tensor_tensor(
                                out=hsv[:, ts - 1:ts, :], in0=hsv[:, ts - 1:ts, :],
                                in1=spad_t[:].unsqueeze(2).broadcast_to([P, 1, H]),
                                op=mybir.AluOpType.add)
                        nc.vector.tensor_copy(
                            dstg[:, :ts * H].rearrange("p (j q) -> p j q", q=H),
                            pdv[:, :, H:SD])
                        rowb = blk * shard_pad + t0 * P
                        nc.sync.dma_start(
                            h_tab[rowb:rowb + ts * P, :].rearrange("(j p) r -> p j r", p=P),
                            hstg[:, :ts * RB].rearrange("p (j r) -> p j r", j=ts))
                        nc.sync.dma_start(
                            dtab[t0 * P:t0 * P + ts * P, blk * H:(blk + 1) * H]
                            .rearrange("(j p) q -> p j q", p=P),
                            dstg[:, :ts * H].rearrange("p (j q) -> p j q", q=H))
                ctx_mm.__exit__(None, None, None)

                # ---- phase 2: edge batches + masked matmul accumulation ----
                ctx_g = nc.named_scope(f"edge{l}"); ctx_g.__enter__()
                oT = sb.tile([P, shard_pad], F32, tag="oT")
                if "edge" in ablate:
                    nc.vector.memset(oT[:], 0.0)
                p_sb = sb.tile([P, NTE * HMAX], BF16, tag="p_sb")
                if "gather" in ablate:
                    nc.vector.memset(p_sb[:], 0.5)
                batches = {}

                def do_batch(reg, b):
                    if (reg, b) in batches:
                        return batches[(reg, b)]
                    r0 = 0 if reg == 0 else NLO
                    rN = NLO if reg == 0 else NTE
                    c0 = r0 + b * GB
                    cs = min(GB, rN - c0)
                    nid = cs * P
                    hg = hgp.tile([P, GB * RB], BF16, tag="hg")
                    dg = dgp.tile([P, GB * DREP], F32, tag="dg")
                    if "gather" not in ablate:
                        in_ap = h_tab[0:LOHI, :] if reg == 0 else h_tab[LOHI:nrows, :]
                        nc.gpsimd.dma_gather(
                            out_ap=hg[:, :cs * RB].rearrange("p (j r) -> p j r", j=cs),
                            in_ap=in_ap, idxs_ap=h16_t[:, c0 * 8:c0 * 8 + cs * 8],
                            num_idxs=nid, num_idxs_reg=nid, elem_size=RB)
                        nc.gpsimd.dma_gather(
                            out_ap=dg[:, :cs * DREP].rearrange("p (j r) -> p j r", j=cs),
                            in_ap=dtab[:], idxs_ap=d16_t[:, c0 * 8:c0 * 8 + cs * 8],
                            num_idxs=nid, num_idxs_reg=nid, elem_size=DREP,
                            queue_num=1)
                    if "gather" in ablate:
                        mk = mkp.tile([P, GB * P], BF16, tag="mk")
                        nc.vector.tensor_tensor(
                            out=mk[:, :cs * P].rearrange("p (j d) -> p j d", j=cs),
                            in0=dl_t[:, c0:c0 + cs].unsqueeze(2).broadcast_to([P, cs, P]),
                            in1=iota_f[:].unsqueeze(1).broadcast_to([P, cs, P]),
                            op=mybir.AluOpType.is_equal)
                        batches[(reg, b)] = (mk, mk)
                        return batches[(reg, b)]
                    # d select: one-hot block mask + pairwise fold (8 blocks)
                    dgv = dg[:, :cs * DREP].rearrange("p (j q) -> p j q", q=DREP)
                    nc.vector.tensor_tensor(
                        out=dgv, in0=dgv,
                        in1=sel_ts[l][:].unsqueeze(1).broadcast_to([P, cs, DREP]),
                        op=mybir.AluOpType.mult)
                    w = NCORES * H
                    while w > H:
                        nc.vector.tensor_tensor(
                            out=dgv[:, :, :w // 2], in0=dgv[:, :, :w // 2],
                            in1=dgv[:, :, w // 2:w], op=mybir.AluOpType.add)
                        w //= 2
                    # e = s + d ; p = exp(leakyrelu(e))
                    et = etp.tile([P, GB * HMAX], F32, tag="et")
                    etv = et[:, :cs * H].rearrange("p (j q) -> p j q", q=H)
                    sv = hg[:, :cs * RB].rearrange("p (j r) -> p j r", j=cs)[
                        :, :, HC:HC + SD].bitcast(F32)
                    nc.vector.tensor_tensor(out=etv, in0=sv, in1=dgv[:, :, :H],
                                            op=mybir.AluOpType.add)
                    nc.vector.scalar_tensor_tensor(
                        out=et[:, :cs * H], in0=et[:, :cs * H], scalar=NEG_SLOPE,
                        in1=et[:, :cs * H],
                        op0=mybir.AluOpType.mult, op1=mybir.AluOpType.max)
                    nc.scalar.activation(p_sb[:, c0 * H:(c0 + cs) * H], et[:, :cs * H],
                                         mybir.ActivationFunctionType.Exp)
                    # in-place alpha scale of the gathered h
                    hv = hg[:, :cs * RB].rearrange("p (j r) -> p j r", j=cs)[
                        :, :, :HC].rearrange("p j (h c) -> p j h c", h=H)
                    nc.vector.tensor_tensor(
                        out=hv, in0=hv,
                        in1=p_sb[:, c0 * H:(c0 + cs) * H]
                        .rearrange("p (j h) -> p j h", j=cs).unsqueeze(3)
                        .broadcast_to([P, cs, H, C]),
                        op=mybir.AluOpType.mult)
                    # one-hot dst masks
                    mk = mkp.tile([P, GB * P], BF16, tag="mk")
                    nc.vector.tensor_tensor(
                        out=mk[:, :cs * P].rearrange("p (j d) -> p j d", j=cs),
                        in0=dl_t[:, c0:c0 + cs].unsqueeze(2).broadcast_to([P, cs, P]),
                        in1=iota_f[:].unsqueeze(1).broadcast_to([P, cs, P]),
                        op=mybir.AluOpType.is_equal)
                    batches[(reg, b)] = (hg, mk)
                    return batches[(reg, b)]

                for T in range(ntiles if "edge" not in ablate else 0):
                    pa = psA.tile([P, 512], F32, tag="acc", space="PSUM")
                    pd = psB.tile([P, EV * 2 * HMAX], F32, tag="sd", space="PSUM")
                    cols = [(0, lo_offs[T] + k) for k in range(K_lo[T])] + \
                           [(1, NLO + hi_offs[T] + k) for k in range(K_hi[T])]
                    nK = len(cols)
                    for k, (reg, c) in enumerate(cols):
                        b, o = divmod(c - (0 if reg == 0 else NLO), GB)
                        hg, mk = do_batch(reg, b)
                        mkap = mk[:, o * P:(o + 1) * P]
                        if "gather" not in ablate:
                            nc.tensor.matmul(pa[:, :HC], lhsT=mkap,
                                             rhs=hg[:, o * RB:o * RB + HC],
                                             start=(k == 0), stop=(k == nK - 1))
                        else:
                            nc.tensor.matmul(pa[:, :HC], lhsT=mkap,
                                             rhs=mk[:, o * P:o * P + P].broadcast_to([P, HC])
                                             if False else mk[:, :HC] if HC <= GB * P else mk[:, :GB * P],
                                             start=(k == 0), stop=(k == nK - 1))
                        nc.tensor.matmul(pd[:, :H], lhsT=mkap,
                                         rhs=p_sb[:, c * H:(c + 1) * H],
                                         start=(k == 0), stop=(k == nK - 1))
                    rcp = smp.tile([P, HMAX], F32, tag="rcp")
                    nc.vector.tensor_scalar_add(rcp[:, :H], pd[:, :H], 1e-16)
                    nc.vector.reciprocal(rcp[:, :H], rcp[:, :H])
                    o_nd = onp.tile([P, 512], F32, tag="o_nd")
                    nc.vector.tensor_tensor(
                        out=o_nd[:, :HC].rearrange("p (h c) -> p h c", h=H),
                        in0=pa[:, :HC].rearrange("p (h c) -> p h c", h=H),
                        in1=rcp[:, :H].unsqueeze(2).broadcast_to([P, H, C]),
                        op=mybir.AluOpType.mult)
                    if H > 1:
                        o_s = onp.tile([P, P], F32, tag="o_s")
                        nc.vector.tensor_tensor(out=o_s[:, :C], in0=o_nd[:, :C],
                                                in1=o_nd[:, C:2 * C],
                                                op=mybir.AluOpType.add)
                        for hh in range(2, H):
                            nc.vector.tensor_tensor(out=o_s[:, :C], in0=o_s[:, :C],
                                                    in1=o_nd[:, hh * C:(hh + 1) * C],
                                                    op=mybir.AluOpType.add)
                        osrc = o_s
                    else:
                        osrc = o_nd
                    ptr = psT.tile([P, P], F32, tag="tr", space="PSUM")
                    nc.tensor.transpose(out=ptr[:], in_=osrc[:, :C], identity=ident[:])
                    nc.vector.tensor_copy(oT[:, T * P:(T + 1) * P], ptr[:])
                ctx_g.__exit__(None, None, None)

                # ---- phase 3: batchnorm (+relu) ----
                ctx_bn = nc.named_scope(f"bn{l}"); ctx_bn.__enter__()
                nsum = smp.tile([P, 1], F32, tag="nsum")
                nsq = smp.tile([P, 1], F32, tag="nsq")
                nc.vector.tensor_reduce(out=nsum[:], in_=oT[:], axis=mybir.AxisListType.X,
                                        op=mybir.AluOpType.add)
                yTn = sb.tile([P, shard_pad], F32, tag="yT")
                nc.scalar.activation(yTn[:], oT[:], mybir.ActivationFunctionType.Square,
                                     accum_out=nsq[:])
                ar_in = dram.tile([P, 2], F32, tag=f"arin{l}")
                ar_out = dram.tile([P, 2], F32, tag=f"arout{l}", addr_space="Shared")
                st2 = smp.tile([P, 2], F32, tag="st2")
                nc.vector.tensor_copy(st2[:, 0:1], nsum[:])
                nc.vector.tensor_copy(st2[:, 1:2], nsq[:])
                nc.gpsimd.dma_start(ar_in[:], st2[:])
                if "coll" not in ablate:
                    nc.gpsimd.collective_compute(
                        "AllReduce", mybir.AluOpType.add,
                        replica_groups=[list(range(NCORES))],
                        ins=[ar_in.opt()], outs=[ar_out.opt()],
                    )
                stg = smp.tile([P, 2], F32, tag="stg")
                nc.sync.dma_start(stg[:], ar_out[:])
                ntotal = float(NCORES * shard)
                mu = smp.tile([P, 1], F32, tag="mu")
                nc.vector.tensor_scalar_mul(mu[:], stg[:, 0:1], 1.0 / ntotal)
                var = smp.tile([P, 1], F32, tag="var")
                nc.vector.tensor_scalar_mul(var[:], stg[:, 1:2], 1.0 / ntotal)
                musq = smp.tile([P, 1], F32, tag="musq")
                nc.vector.tensor_tensor(out=musq[:], in0=mu[:], in1=mu[:], op=mybir.AluOpType.mult)
                nc.vector.tensor_tensor(out=var[:], in0=var[:], in1=musq[:], op=mybir.AluOpType.subtract)
                rstd = smp.tile([P, 1], F32, tag="rstd")
                nc.vector.tensor_scalar_add(var[:], var[:], EPS_BN)
                nc.scalar.activation(rstd[:], var[:], mybir.ActivationFunctionType.Sqrt)
                nc.vector.reciprocal(rstd[:], rstd[:])
                scale = smp.tile([P, 1], F32, tag="scale")
                nc.vector.tensor_tensor(out=scale[:], in0=gb_t[:, 2 * l:2 * l + 1], in1=rstd[:],
                                        op=mybir.AluOpType.mult)
                shift = smp.tile([P, 1], F32, tag="shift")
                nc.vector.tensor_tensor(out=shift[:], in0=mu[:], in1=scale[:], op=mybir.AluOpType.mult)
                nc.vector.tensor_tensor(out=shift[:], in0=gb_t[:, 2 * l + 1:2 * l + 2], in1=shift[:],
                                        op=mybir.AluOpType.subtract)
                func = (mybir.ActivationFunctionType.Relu if l < n_l - 1
                        else mybir.ActivationFunctionType.Identity)
                nc.scalar.activation(yTn[:], oT[:], func, bias=shift[:], scale=scale[:])
                npad = shard_pad - shard
                if npad > 0 and l < n_l - 1:
                    nc.vector.memset(yTn[:, shard:], 0.0)
                yT = yTn
                ctx_bn.__exit__(None, None, None)

            out_b = sb.tile([P, shard_pad], BF16, tag="out_b")
            nc.vector.tensor_copy(out_b[:], yT[:])
            nc.sync.dma_start(out_t[:], out_b[:])

    nc.compile()
    return nc


# ----------------------------------------------------------------------------
# entry point
# ----------------------------------------------------------------------------

def build_for_inputs(x, edge_index, params_list, ablate=(), nlayers=3):
    """params_list = [(W, a_src, a_dst, gamma, beta), ...]"""
    x = np.asarray(x, np.float32)
    N, in_dim = x.shape
    g = _prep(np.asarray(edge_index), N)
    params = params_list[:nlayers]
    layers = []
    for (W, asr, ads, gmm, bet) in params:
        H, C = asr.shape
        layers.append({"H": H, "C": C})
    nc = _build_program(g, layers, in_dim, ablate=ablate)

    wexts = []
    for l, ((W, asr, ads, gmm, bet), L) in enumerate(zip(params, layers)):
        H, C = L["H"], L["C"]
        w_s = np.einsum("khc,hc->kh", W.reshape(W.shape[0], H, C), asr)
        w_d = np.einsum("khc,hc->kh", W.reshape(W.shape[0], H, C), ads)
        we = np.concatenate([W, w_s, w_d], axis=1)
        wexts.append(we.astype(mybir.dt.np(BF16)))
    gbm = np.zeros((P, 2 * len(layers)), np.float32)
    for l, (W, asr, ads, gmm, bet) in enumerate(params):
        gbm[:len(gmm), 2 * l] = gmm
        gbm[:len(bet), 2 * l + 1] = bet

    shard, shard_pad, nrows = g["shard"], g["shard_pad"], g["nrows"]
    ntiles = g["ntiles"]
    bt = mybir.dt.np(BF16)
    xTs = []
    for c in range(NCORES):
        xc = np.zeros((in_dim, shard_pad), bt)
        xc[:, :shard] = x[g["out_nodes"][c]].T.astype(bt)
        xTs.append(xc)
    spad = np.zeros((P, 1), np.float32)
    lastbase = (ntiles - 1) * P
    for p in range(P):
        if lastbase + p >= shard:
            spad[p, 0] = -1e30

    in_maps = []
    for c in range(NCORES):
        m = {"xT": xTs[c], "h16": np.ascontiguousarray(g["h16"][c][:16]),
             "d16": np.ascontiguousarray(g["d16"][c][:16]),
             "dstloc": np.ascontiguousarray(g["dstloc"][c].astype(np.int8)),
             "gb": gbm, "spad": spad}
        for l, L in enumerate(layers):
            sel = np.zeros((P, DREP), np.float32)
            sel[:, c * L["H"]:(c + 1) * L["H"]] = 1.0
            m[f"sel{l}"] = sel
        for l, w in enumerate(wexts):
            m[f"wext{l}"] = w
        in_maps.append(m)
    return nc, in_maps, g, layers


def kernel(x, edge_index,
           W0, a_src0, a_dst0, b0, gamma0, beta0,
           W1, a_src1, a_dst1, b1, gamma1, beta1,
           W2, a_src2, a_dst2, b2, gamma2, beta2, _profile=None, _nlayers=3,
           _ablate=()):
    params = [(np.asarray(W0, np.float32), np.asarray(a_src0, np.float32), np.asarray(a_dst0, np.float32),
               np.asarray(gamma0, np.float32), np.asarray(beta0, np.float32)),
              (np.asarray(W1, np.float32), np.asarray(a_src1, np.float32), np.asarray(a_dst1, np.float32),
               np.asarray(gamma1, np.float32), np.asarray(beta1, np.float32)),
              (np.asarray(W2, np.float32), np.asarray(a_src2, np.float32), np.asarray(a_dst2, np.float32),
               np.asarray(gamma2, np.float32), np.asarray(beta2, np.float32))]
    nc, in_maps, g, layers = build_for_inputs(x, edge_index, params,
                                              ablate=_ablate, nlayers=_nlayers)
    if _profile is not None:
        _profile["nc"] = nc
        _profile["in_maps"] = in_maps
    res = bass_utils.run_bass_kernel_spmd(nc, in_maps, core_ids=list(range(NCORES)))

    N = np.asarray(x).shape[0]
    C_out = layers[-1]["C"]
    shard = g["shard"]
    out = np.empty((N, C_out), np.float32)
    for c in range(NCORES):
        yT = res.results[c]["out"].astype(np.float32)   # [P(feat), shard_pad]
        out[g["out_nodes"][c]] = yT[:C_out, :shard].T
    if _profile is not None:
        _profile["results"] = res
    return out
